# revision 2
# baseline (speedup 1.0000x reference)
"""Ternary-expert MLP (gate/up/silu/down) on 8 trn2 NeuronCores.

Strategy: data-parallel over tokens (512/core), hybrid precision.

The per-channel dequant scales gate_s*up_s span ~100x, so output error is
dominated by the loudest intermediate channels. Channels are permuted
host-side by descending gate_s*up_s: the loudest NL i-tiles run the exact
fp16 path (scales folded into weights, like the fp16 baseline); the quiet
NQ i-tiles run fp8-e4m3 DoubleRow matmuls (2 k-tiles per instruction,
~1.4-2x PE throughput) with EXACT ternary {-1,0,+1} fp8 weights and the
scales applied post-matmul (sigmoid's scale operand + tensor_scalar ops).
Only fp8(x) and fp8(hdn_quiet) quantization contribute error; simulated
absmax rel err at NL=18 is 1.4e-2 vs the 2e-2 gate.

Per core, everything is computed in "transposed" space so the contraction
dim always sits on SBUF partitions:
  phase 1: for each inter tile  g^T/u^T [128i, 512t] = W-tile.T @ x^T
           hdn^T = silu(g^T) * u^T   (fp16 loud / fp8 quiet, stays in SBUF)
  phase 2: for each of 16 hidden tiles out^T [128h, 512t] = D-tile.T @ hdn^T
"""

import numpy as np
import ml_dtypes

HIDDEN = 2048
INTER = 5632
B, S = 2, 2048
T = B * S
NCORES = 8
TPC = T // NCORES          # 512 tokens per core
P = 128
KH = HIDDEN // P           # 16 hidden-dim k-tiles
NI = INTER // P            # 44 intermediate tiles
NL = 18                    # loud (fp16) i-tiles
NQ = NI - NL               # quiet (fp8 DoubleRow) i-tiles; must be even

_cache = {}


def _build_nc(kh=KH, nl=NL, nq=NQ, tpc=TPC):
    import concourse.bacc as bacc
    import concourse.tile as tile
    from concourse import mybir

    f16 = mybir.dt.float16
    f32 = mybir.dt.float32
    f8 = mybir.dt.float8e4
    DR = mybir.MatmulPerfMode.DoubleRow

    nc = bacc.Bacc("TRN2", target_bir_lowering=False, debug=False)
    xt = nc.dram_tensor("xt", [P, kh, tpc], f16, kind="ExternalInput").ap()
    xt8 = nc.dram_tensor("xt8", [P, kh, tpc], f8, kind="ExternalInput").ap()
    gwl = nc.dram_tensor("gwl", [nl, P, kh * P], f16, kind="ExternalInput").ap()
    uwl = nc.dram_tensor("uwl", [nl, P, kh * P], f16, kind="ExternalInput").ap()
    gwq = nc.dram_tensor("gwq", [nq, P, kh * P], f8, kind="ExternalInput").ap()
    uwq = nc.dram_tensor("uwq", [nq, P, kh * P], f8, kind="ExternalInput").ap()
    dwl = nc.dram_tensor("dwl", [kh, P, nl * P], f16, kind="ExternalInput").ap()
    dwq = nc.dram_tensor("dwq", [kh, P, nq * P], f8, kind="ExternalInput").ap()
    gsq = nc.dram_tensor("gsq", [P, nq], f32, kind="ExternalInput").ap()
    guq = nc.dram_tensor("guq", [P, nq], f32, kind="ExternalInput").ap()
    dst = nc.dram_tensor("dst", [P, kh], f32, kind="ExternalInput").ap()
    ot = nc.dram_tensor("ot", [kh, P, tpc], f32, kind="ExternalOutput").ap()

    with tile.TileContext(nc) as tc:
        with (
            tc.tile_pool(name="xp", bufs=1) as xp,
            tc.tile_pool(name="hp", bufs=1) as hp,
            tc.tile_pool(name="sp", bufs=1) as sp,
            tc.tile_pool(name="wg", bufs=3) as wg_pool,
            tc.tile_pool(name="wu", bufs=3) as wu_pool,
            tc.tile_pool(name="wd", bufs=3) as wd_pool,
            tc.tile_pool(name="act", bufs=3) as act_pool,
            tc.tile_pool(name="ob", bufs=3) as ob_pool,
            tc.tile_pool(name="ps", bufs=2, space="PSUM") as ps_pool,
            tc.tile_pool(name="po", bufs=2, space="PSUM") as po_pool,
        ):
            # PE warmup: HAM un-throttles only after ~3.4us of sustained PE
            # activity; bridge the initial DMA wait with dummy matmuls on a
            # zeroed tile so the real stream starts at 2.4 GHz.
            wz = act_pool.tile([P, P], f16, tag="warm")
            nc.vector.memset(wz[:], 0.0)
            pw = po_pool.tile([P, P], f32, tag="warmp")
            for _ in range(34):
                nc.tensor.matmul(pw[:], wz[:], wz[:], start=True, stop=True)

            xsb = xp.tile([P, kh, tpc], f16)
            x8sb = xp.tile([P, kh, tpc], f8, tag="x8")
            hdl = hp.tile([P, nl, tpc], f16, tag="hl")
            hdq = hp.tile([P, nq, tpc], f8, tag="hq")
            gsq_sb = sp.tile([P, nq], f32, tag="gsq")
            guq_sb = sp.tile([P, nq], f32, tag="guq")
            dst_sb = sp.tile([P, kh], f32, tag="dst")
            nc.sync.dma_start(out=gsq_sb[:], in_=gsq)
            nc.sync.dma_start(out=guq_sb[:], in_=guq)
            nc.sync.dma_start(out=dst_sb[:], in_=dst)

            # issue the first loads in consumption order: gate slab 0, first
            # quarter of x, up slab 0, rest of x, then fp8 x
            ck = max(1, kh // 4)
            bounds = [(s0, min(s0 + ck, kh)) for s0 in range(0, kh, ck)]
            wgt0 = wg_pool.tile([P, kh, P], f16, tag="wgt")
            nc.sync.dma_start(out=wgt0[:], in_=gwl[0])
            nc.sync.dma_start(out=xsb[:, bounds[0][0]:bounds[0][1]],
                              in_=xt[:, bounds[0][0]:bounds[0][1]])
            wut0 = wu_pool.tile([P, kh, P], f16, tag="wut")
            nc.sync.dma_start(out=wut0[:], in_=uwl[0])
            for lo, hi in bounds[1:]:
                nc.sync.dma_start(out=xsb[:, lo:hi], in_=xt[:, lo:hi])
            nc.sync.dma_start(out=x8sb[:], in_=xt8)

            for it in range(nl + nq):
                if it < nl:
                    # ---- loud fp16 path (scales folded in weights) ----
                    if it == 0:
                        wgt, wut = wgt0, wut0
                    else:
                        wgt = wg_pool.tile([P, kh, P], f16, tag="wgt")
                        nc.sync.dma_start(out=wgt[:], in_=gwl[it])
                        wut = wu_pool.tile([P, kh, P], f16, tag="wut")
                        nc.sync.dma_start(out=wut[:], in_=uwl[it])
                    pg = ps_pool.tile([P, tpc], f32)
                    pu = ps_pool.tile([P, tpc], f32)
                    for k in range(kh):
                        nc.tensor.matmul(
                            pg[:], wgt[:, k], xsb[:, k],
                            start=(k == 0), stop=(k == kh - 1),
                        )
                    for k in range(kh):
                        nc.tensor.matmul(
                            pu[:], wut[:, k], xsb[:, k],
                            start=(k == 0), stop=(k == kh - 1),
                        )
                    sg = act_pool.tile([P, tpc], f16)
                    nc.scalar.activation(sg[:], pg[:],
                                         mybir.ActivationFunctionType.Sigmoid)
                    sm = act_pool.tile([P, tpc], f16)
                    nc.vector.tensor_mul(sm[:], sg[:], pg[:])
                    nc.vector.tensor_mul(hdl[:, it], sm[:], pu[:])
                else:
                    # ---- quiet fp8 DoubleRow path (exact ternary weights,
                    # scales applied post-matmul) ----
                    iq = it - nl
                    wgt = wg_pool.tile([P, kh, P], f8, tag="wgt8")
                    nc.sync.dma_start(out=wgt[:], in_=gwq[iq])
                    wut = wu_pool.tile([P, kh, P], f8, tag="wut8")
                    nc.sync.dma_start(out=wut[:], in_=uwq[iq])
                    pg = ps_pool.tile([P, tpc], f32)
                    pu = ps_pool.tile([P, tpc], f32)
                    for k in range(0, kh, 2):
                        nc.tensor.matmul(
                            pg[:], wgt[:, k:k + 2], x8sb[:, k:k + 2],
                            start=(k == 0), stop=(k == kh - 2), perf_mode=DR,
                        )
                    for k in range(0, kh, 2):
                        nc.tensor.matmul(
                            pu[:], wut[:, k:k + 2], x8sb[:, k:k + 2],
                            start=(k == 0), stop=(k == kh - 2), perf_mode=DR,
                        )
                    # hdn = silu(gs*g)*us*u = sigmoid(gs*g) * g * (gs*us*u)
                    sg = act_pool.tile([P, tpc], f16)
                    nc.scalar.activation(sg[:], pg[:],
                                         mybir.ActivationFunctionType.Sigmoid,
                                         scale=gsq_sb[:, iq:iq + 1])
                    tq = act_pool.tile([P, tpc], f16)
                    nc.vector.tensor_mul(tq[:], sg[:], pg[:])
                    usc = act_pool.tile([P, tpc], f16)
                    nc.vector.tensor_scalar_mul(usc[:], pu[:],
                                                guq_sb[:, iq:iq + 1])
                    nc.vector.tensor_mul(hdq[:, iq], tq[:], usc[:])

            for hg in range(kh):
                wdt = wd_pool.tile([P, nl, P], f16, tag="wdl")
                nc.sync.dma_start(out=wdt[:], in_=dwl[hg])
                wdt8 = wd_pool.tile([P, nq, P], f8, tag="wdq")
                nc.sync.dma_start(out=wdt8[:], in_=dwq[hg])
                po = po_pool.tile([P, tpc], f32)
                for il in range(nl):
                    nc.tensor.matmul(
                        po[:], wdt[:, il], hdl[:, il],
                        start=(il == 0), stop=False,
                    )
                for qj in range(0, nq, 2):
                    nc.tensor.matmul(
                        po[:], wdt8[:, qj:qj + 2], hdq[:, qj:qj + 2],
                        start=False, stop=(qj == nq - 2), perf_mode=DR,
                    )
                ob = ob_pool.tile([P, tpc], f32)
                nc.vector.tensor_scalar_mul(ob[:], po[:], dst_sb[:, hg:hg + 1])
                nc.sync.dma_start(out=ot[hg], in_=ob[:])

    nc.compile()
    return nc


def _pack_weights(gate_w, up_w, down_w, gate_s, up_s, down_s):
    f8 = ml_dtypes.float8_e4m3
    perm = np.argsort(-(gate_s * up_s), kind="stable")
    gw = gate_w[perm]
    uw = up_w[perm]
    dw = down_w[:, perm]
    gs = gate_s[perm]
    us = up_s[perm]
    nlp = NL * P

    def pack_gu(w, s, n, dtype, fold):
        wp = w * s[:, None] if fold else w
        wp = wp.reshape(n, P, KH, P).transpose(0, 3, 2, 1)
        return np.ascontiguousarray(wp).astype(dtype).reshape(n, P, KH * P)

    gwl = pack_gu(gw[:nlp], gs[:nlp], NL, np.float16, True)
    uwl = pack_gu(uw[:nlp], us[:nlp], NL, np.float16, True)
    gwq = pack_gu(gw[nlp:], None, NQ, f8, False)
    uwq = pack_gu(uw[nlp:], None, NQ, f8, False)

    def pack_d(w, n, dtype):
        wp = w.reshape(KH, P, n, P).transpose(0, 3, 2, 1)
        return np.ascontiguousarray(wp).astype(dtype).reshape(KH, P, n * P)

    dwl = pack_d(dw[:, :nlp], NL, np.float16)
    dwq = pack_d(dw[:, nlp:], NQ, f8)

    gsq = np.ascontiguousarray(gs[nlp:].reshape(NQ, P).T)
    guq = np.ascontiguousarray((gs * us)[nlp:].reshape(NQ, P).T)
    dst = np.ascontiguousarray(down_s.reshape(KH, P).T)
    return dict(gwl=gwl, uwl=uwl, gwq=gwq, uwq=uwq, dwl=dwl, dwq=dwq,
                gsq=gsq, guq=guq, dst=dst)


def _pack_x(xf):
    # per-core x^T tiles: xt[p, k, t] = x_core[t, k*128+p]
    outs = []
    for c in range(NCORES):
        xc = xf[c * TPC:(c + 1) * TPC].reshape(TPC, KH, P)
        xc = np.ascontiguousarray(xc.transpose(2, 1, 0))
        outs.append((xc.astype(np.float16),
                     xc.astype(ml_dtypes.float8_e4m3)))
    return outs


def _ensure_ntff_hook():
    """bass_utils' axon trace path imports antenv.axon_hooks, which is
    missing from this image; provide it (ctypes into libaxon_pjrt.so) so a
    BASS_TRACE=1 environment doesn't crash the run."""
    import sys
    try:
        import antenv.axon_hooks  # noqa: F401
        return
    except ImportError:
        pass
    import contextlib
    import ctypes
    import types

    def _make_hook():
        try:
            lib = ctypes.CDLL("/opt/axon/libaxon_pjrt.so")
            lib.axon_start_nrt_profile
        except Exception:
            return None
        lib.axon_start_nrt_profile.argtypes = [ctypes.POINTER(ctypes.c_int64),
                                               ctypes.c_size_t]
        lib.axon_start_nrt_profile.restype = ctypes.c_int64
        lib.axon_stop_nrt_profile.argtypes = [ctypes.c_char_p]
        lib.axon_stop_nrt_profile.restype = ctypes.c_int64

        @contextlib.contextmanager
        def _hook(output_dir, device_ids):
            import jax
            jax.devices()
            if device_ids:
                ids = (ctypes.c_int64 * len(device_ids))(*device_ids)
                rc = lib.axon_start_nrt_profile(ids, len(device_ids))
            else:
                rc = lib.axon_start_nrt_profile(None, 0)
            if rc != 0:
                raise RuntimeError(f"axon_start_nrt_profile rc={rc}")
            try:
                yield
            finally:
                lib.axon_stop_nrt_profile(str(output_dir).encode())

        return _hook

    mod = types.ModuleType("antenv.axon_hooks")
    _hook = _make_hook()
    mod.get_axon_ntff_profile_hook = lambda: _hook
    mod.set_axon_ntff_profile_hook = lambda h: None
    sys.modules["antenv.axon_hooks"] = mod


def _run(in_maps, trace=False, tmpdir=None, trace_cores=None):
    from concourse.bass_utils import run_bass_kernel_spmd

    _ensure_ntff_hook()
    if "nc" not in _cache:
        _cache["nc"] = _build_nc()
    return run_bass_kernel_spmd(
        _cache["nc"], in_maps, list(range(NCORES)), trace=trace, tmpdir=tmpdir,
        trace_cores=trace_cores,
    )


def make_in_maps(x, gate_w, up_w, down_w, gate_s, up_s, down_s):
    x = np.asarray(x, np.float32)
    gate_w = np.asarray(gate_w, np.float32)
    up_w = np.asarray(up_w, np.float32)
    down_w = np.asarray(down_w, np.float32)
    gate_s = np.asarray(gate_s, np.float32)
    up_s = np.asarray(up_s, np.float32)
    down_s = np.asarray(down_s, np.float32)

    wmap = _pack_weights(gate_w, up_w, down_w, gate_s, up_s, down_s)
    xts = _pack_x(x.reshape(T, HIDDEN))
    return [dict(xt=xts[c][0], xt8=xts[c][1], **wmap) for c in range(NCORES)]


def unpack_out(results):
    # ot: (16, 128, 512) f32 per core; out_core[t, hg*128+p] = ot[hg, p, t]
    parts = []
    for c in range(NCORES):
        ot = results[c]["ot"]
        parts.append(ot.transpose(2, 0, 1).reshape(TPC, HIDDEN))
    return np.concatenate(parts, axis=0).reshape(B, S, HIDDEN)


def kernel(x, gate_w, up_w, down_w, gate_s, up_s, down_s):
    in_maps = make_in_maps(x, gate_w, up_w, down_w, gate_s, up_s, down_s)
    try:
        res = _run(in_maps)
    except Exception:
        # transient runtime errors (device hiccup) — one retry
        res = _run(in_maps)
    return unpack_out(res.results)


# revision 4
# speedup vs baseline: 1.0326x; 1.0326x over previous
"""Ternary-expert MLP (gate/up/silu/down) on 8 trn2 NeuronCores.

Strategy: data-parallel over tokens (512/core), hybrid precision.

All weights ship as EXACT ternary {-1,0,+1} fp8-e4m3 (1 byte) with the
per-channel dequant scales applied post-matmul (sigmoid's scale operand +
tensor_scalar ops) — the PE supports mixed fp8-stationary x fp16-moving
matmuls at full fp16 speed, verified bit-exact on HW.

The scales gate_s*up_s span ~100x, so output error is dominated by the
loudest intermediate channels. Channels are permuted host-side by
descending gate_s*up_s: the loudest NL=16 i-tiles keep x and hdn in fp16
(error ~= fp16 baseline); the quiet NQ=28 i-tiles use fp8(x)/fp8(hdn)
with DoubleRow matmuls (2 k-tiles per instruction = 2x PE throughput).
Simulated absmax rel err 1.64e-2 vs the 2e-2 gate (HW matched sim to
5e-5 at NL=18).

Per core, everything is computed in "transposed" space so the contraction
dim always sits on SBUF partitions:
  phase 1: for each inter tile  g^T/u^T [128i, 512t] = W-tile.T @ x^T
           hdn^T = sigmoid(gs*g)*g * (gs*us*u)  (fp16 loud / fp8 quiet)
  phase 2: for each of 16 hidden tiles out^T [128h, 512t] = D-tile.T @ hdn^T
           scaled by down_s per partition.
"""

import numpy as np
import ml_dtypes

HIDDEN = 2048
INTER = 5632
B, S = 2, 2048
T = B * S
NCORES = 8
TPC = T // NCORES          # 512 tokens per core
P = 128
KH = HIDDEN // P           # 16 hidden-dim k-tiles
NI = INTER // P            # 44 intermediate tiles
NL = 16                    # loud (fp16 activations) i-tiles
NQ = NI - NL               # quiet (fp8 DoubleRow) i-tiles; must be even

_cache = {}


def _build_nc(kh=KH, nl=NL, nq=NQ, ni=NI, tpc=TPC):
    import concourse.bacc as bacc
    import concourse.tile as tile
    from concourse import mybir

    f16 = mybir.dt.float16
    f32 = mybir.dt.float32
    f8 = mybir.dt.float8e4
    DR = mybir.MatmulPerfMode.DoubleRow

    nc = bacc.Bacc("TRN2", target_bir_lowering=False, debug=False)
    xt = nc.dram_tensor("xt", [P, kh, tpc], f16, kind="ExternalInput").ap()
    xt8 = nc.dram_tensor("xt8", [P, kh, tpc], f8, kind="ExternalInput").ap()
    gw8 = nc.dram_tensor("gw8", [ni, P, kh * P], f8, kind="ExternalInput").ap()
    uw8 = nc.dram_tensor("uw8", [ni, P, kh * P], f8, kind="ExternalInput").ap()
    dw8 = nc.dram_tensor("dw8", [kh, P, ni * P], f8, kind="ExternalInput").ap()
    gsa = nc.dram_tensor("gsa", [P, ni], f32, kind="ExternalInput").ap()
    gua = nc.dram_tensor("gua", [P, ni], f32, kind="ExternalInput").ap()
    dst = nc.dram_tensor("dst", [P, kh], f32, kind="ExternalInput").ap()
    ot = nc.dram_tensor("ot", [kh, P, tpc], f32, kind="ExternalOutput").ap()

    with tile.TileContext(nc) as tc:
        with (
            tc.tile_pool(name="xp", bufs=1) as xp,
            tc.tile_pool(name="hp", bufs=1) as hp,
            tc.tile_pool(name="sp", bufs=1) as sp,
            tc.tile_pool(name="wg", bufs=3) as wg_pool,
            tc.tile_pool(name="wu", bufs=3) as wu_pool,
            tc.tile_pool(name="wd", bufs=4) as wd_pool,
            tc.tile_pool(name="act", bufs=3) as act_pool,
            tc.tile_pool(name="ob", bufs=3) as ob_pool,
            tc.tile_pool(name="ps", bufs=2, space="PSUM") as ps_pool,
            tc.tile_pool(name="po", bufs=2, space="PSUM") as po_pool,
        ):
            # PE warmup: HAM un-throttles only after ~3.4us of sustained PE
            # activity; bridge the initial DMA wait with dummy matmuls on a
            # zeroed tile so the real stream starts at 2.4 GHz.
            wz = act_pool.tile([P, P], f16, tag="warm")
            nc.vector.memset(wz[:], 0.0)
            pw = po_pool.tile([P, P], f32, tag="warmp")
            for _ in range(34):
                nc.tensor.matmul(pw[:], wz[:], wz[:], start=True, stop=True)

            xsb = xp.tile([P, kh, tpc], f16)
            x8sb = xp.tile([P, kh, tpc], f8, tag="x8")
            hdl = hp.tile([P, nl, tpc], f16, tag="hl")
            hdq = hp.tile([P, nq, tpc], f8, tag="hq")
            gsa_sb = sp.tile([P, ni], f32, tag="gsa")
            gua_sb = sp.tile([P, ni], f32, tag="gua")
            dst_sb = sp.tile([P, kh], f32, tag="dst")
            nc.sync.dma_start(out=gsa_sb[:], in_=gsa)
            nc.sync.dma_start(out=gua_sb[:], in_=gua)
            nc.sync.dma_start(out=dst_sb[:], in_=dst)

            # issue the first loads in consumption order: gate slab 0, first
            # quarter of x, up slab 0, rest of x, then fp8 x
            ck = max(1, kh // 4)
            bounds = [(s0, min(s0 + ck, kh)) for s0 in range(0, kh, ck)]
            wgt0 = wg_pool.tile([P, kh, P], f8, tag="wgt")
            nc.sync.dma_start(out=wgt0[:], in_=gw8[0])
            nc.sync.dma_start(out=xsb[:, bounds[0][0]:bounds[0][1]],
                              in_=xt[:, bounds[0][0]:bounds[0][1]])
            wut0 = wu_pool.tile([P, kh, P], f8, tag="wut")
            nc.sync.dma_start(out=wut0[:], in_=uw8[0])
            for lo, hi in bounds[1:]:
                nc.sync.dma_start(out=xsb[:, lo:hi], in_=xt[:, lo:hi])
            nc.sync.dma_start(out=x8sb[:], in_=xt8)

            for it in range(ni):
                if it == 0:
                    wgt, wut = wgt0, wut0
                else:
                    wgt = wg_pool.tile([P, kh, P], f8, tag="wgt")
                    nc.sync.dma_start(out=wgt[:], in_=gw8[it])
                    wut = wu_pool.tile([P, kh, P], f8, tag="wut")
                    nc.sync.dma_start(out=wut[:], in_=uw8[it])
                pg = ps_pool.tile([P, tpc], f32)
                pu = ps_pool.tile([P, tpc], f32)
                if it < nl:
                    # loud: fp16 x moving, fp8 ternary stationary (exact)
                    for k in range(kh):
                        nc.tensor.matmul(
                            pg[:], wgt[:, k], xsb[:, k],
                            start=(k == 0), stop=(k == kh - 1),
                        )
                    for k in range(kh):
                        nc.tensor.matmul(
                            pu[:], wut[:, k], xsb[:, k],
                            start=(k == 0), stop=(k == kh - 1),
                        )
                else:
                    # quiet: fp8 x, DoubleRow over k-tile pairs
                    for k in range(0, kh, 2):
                        nc.tensor.matmul(
                            pg[:], wgt[:, k:k + 2], x8sb[:, k:k + 2],
                            start=(k == 0), stop=(k == kh - 2), perf_mode=DR,
                        )
                    for k in range(0, kh, 2):
                        nc.tensor.matmul(
                            pu[:], wut[:, k:k + 2], x8sb[:, k:k + 2],
                            start=(k == 0), stop=(k == kh - 2), perf_mode=DR,
                        )
                # hdn = silu(gs*g)*us*u = sigmoid(gs*g) * g * (gs*us*u)
                sg = act_pool.tile([P, tpc], f16)
                nc.scalar.activation(sg[:], pg[:],
                                     mybir.ActivationFunctionType.Sigmoid,
                                     scale=gsa_sb[:, it:it + 1])
                tq = act_pool.tile([P, tpc], f16)
                nc.vector.tensor_mul(tq[:], sg[:], pg[:])
                usc = act_pool.tile([P, tpc], f16)
                nc.vector.tensor_scalar_mul(usc[:], pu[:], gua_sb[:, it:it + 1])
                if it < nl:
                    nc.vector.tensor_mul(hdl[:, it], tq[:], usc[:])
                else:
                    nc.vector.tensor_mul(hdq[:, it - nl], tq[:], usc[:])

            for hg in range(kh):
                wdt = wd_pool.tile([P, ni, P], f8, tag="wd")
                nc.sync.dma_start(out=wdt[:], in_=dw8[hg])
                po = po_pool.tile([P, tpc], f32)
                for il in range(nl):
                    nc.tensor.matmul(
                        po[:], wdt[:, il], hdl[:, il],
                        start=(il == 0), stop=False,
                    )
                for qj in range(0, nq, 2):
                    nc.tensor.matmul(
                        po[:], wdt[:, nl + qj:nl + qj + 2], hdq[:, qj:qj + 2],
                        start=False, stop=(qj == nq - 2), perf_mode=DR,
                    )
                ob = ob_pool.tile([P, tpc], f32)
                nc.vector.tensor_scalar_mul(ob[:], po[:], dst_sb[:, hg:hg + 1])
                nc.sync.dma_start(out=ot[hg], in_=ob[:])

    nc.compile()
    return nc


def _pack_weights(gate_w, up_w, down_w, gate_s, up_s, down_s):
    f8 = ml_dtypes.float8_e4m3
    perm = np.argsort(-(gate_s * up_s), kind="stable")
    gw = gate_w[perm]
    uw = up_w[perm]
    dw = down_w[:, perm]
    gs = gate_s[perm]
    us = up_s[perm]

    def pack_gu(w):
        wp = w.reshape(NI, P, KH, P).transpose(0, 3, 2, 1)
        return np.ascontiguousarray(wp).astype(f8).reshape(NI, P, KH * P)

    gw8 = pack_gu(gw)
    uw8 = pack_gu(uw)
    dwp = dw.reshape(KH, P, NI, P).transpose(0, 3, 2, 1)
    dw8 = np.ascontiguousarray(dwp).astype(f8).reshape(KH, P, NI * P)

    gsa = np.ascontiguousarray(gs.reshape(NI, P).T)
    gua = np.ascontiguousarray((gs * us).reshape(NI, P).T)
    dst = np.ascontiguousarray(down_s.reshape(KH, P).T)
    return dict(gw8=gw8, uw8=uw8, dw8=dw8, gsa=gsa, gua=gua, dst=dst)


def _pack_x(xf):
    # per-core x^T tiles: xt[p, k, t] = x_core[t, k*128+p]
    outs = []
    for c in range(NCORES):
        xc = xf[c * TPC:(c + 1) * TPC].reshape(TPC, KH, P)
        xc = np.ascontiguousarray(xc.transpose(2, 1, 0))
        outs.append((xc.astype(np.float16),
                     xc.astype(ml_dtypes.float8_e4m3)))
    return outs


def _ensure_ntff_hook():
    """bass_utils' axon trace path imports antenv.axon_hooks, which is
    missing from this image; provide it (ctypes into libaxon_pjrt.so) so a
    BASS_TRACE=1 environment doesn't crash the run."""
    import sys
    try:
        import antenv.axon_hooks  # noqa: F401
        return
    except ImportError:
        pass
    import contextlib
    import ctypes
    import types

    def _make_hook():
        try:
            lib = ctypes.CDLL("/opt/axon/libaxon_pjrt.so")
            lib.axon_start_nrt_profile
        except Exception:
            return None
        lib.axon_start_nrt_profile.argtypes = [ctypes.POINTER(ctypes.c_int64),
                                               ctypes.c_size_t]
        lib.axon_start_nrt_profile.restype = ctypes.c_int64
        lib.axon_stop_nrt_profile.argtypes = [ctypes.c_char_p]
        lib.axon_stop_nrt_profile.restype = ctypes.c_int64

        @contextlib.contextmanager
        def _hook(output_dir, device_ids):
            import jax
            jax.devices()
            if device_ids:
                ids = (ctypes.c_int64 * len(device_ids))(*device_ids)
                rc = lib.axon_start_nrt_profile(ids, len(device_ids))
            else:
                rc = lib.axon_start_nrt_profile(None, 0)
            if rc != 0:
                raise RuntimeError(f"axon_start_nrt_profile rc={rc}")
            try:
                yield
            finally:
                lib.axon_stop_nrt_profile(str(output_dir).encode())

        return _hook

    mod = types.ModuleType("antenv.axon_hooks")
    _hook = _make_hook()
    mod.get_axon_ntff_profile_hook = lambda: _hook
    mod.set_axon_ntff_profile_hook = lambda h: None
    sys.modules["antenv.axon_hooks"] = mod


def _run(in_maps, trace=False, tmpdir=None, trace_cores=None):
    from concourse.bass_utils import run_bass_kernel_spmd

    _ensure_ntff_hook()
    if "nc" not in _cache:
        _cache["nc"] = _build_nc()
    return run_bass_kernel_spmd(
        _cache["nc"], in_maps, list(range(NCORES)), trace=trace, tmpdir=tmpdir,
        trace_cores=trace_cores,
    )


def make_in_maps(x, gate_w, up_w, down_w, gate_s, up_s, down_s):
    x = np.asarray(x, np.float32)
    gate_w = np.asarray(gate_w, np.float32)
    up_w = np.asarray(up_w, np.float32)
    down_w = np.asarray(down_w, np.float32)
    gate_s = np.asarray(gate_s, np.float32)
    up_s = np.asarray(up_s, np.float32)
    down_s = np.asarray(down_s, np.float32)

    wmap = _pack_weights(gate_w, up_w, down_w, gate_s, up_s, down_s)
    xts = _pack_x(x.reshape(T, HIDDEN))
    return [dict(xt=xts[c][0], xt8=xts[c][1], **wmap) for c in range(NCORES)]


def unpack_out(results):
    # ot: (16, 128, 512) f32 per core; out_core[t, hg*128+p] = ot[hg, p, t]
    parts = []
    for c in range(NCORES):
        ot = results[c]["ot"]
        parts.append(ot.transpose(2, 0, 1).reshape(TPC, HIDDEN))
    return np.concatenate(parts, axis=0).reshape(B, S, HIDDEN)


def kernel(x, gate_w, up_w, down_w, gate_s, up_s, down_s):
    in_maps = make_in_maps(x, gate_w, up_w, down_w, gate_s, up_s, down_s)
    try:
        res = _run(in_maps)
    except Exception:
        # transient runtime errors (device hiccup) — one retry
        res = _run(in_maps)
    return unpack_out(res.results)


# revision 12
# speedup vs baseline: 1.0340x; 1.0014x over previous
"""Ternary-expert MLP (gate/up/silu/down) on 8 trn2 NeuronCores.

Strategy: data-parallel over tokens (512/core), hybrid precision.

All weights ship as EXACT ternary {-1,0,+1} fp8-e4m3 (1 byte) with the
per-channel dequant scales applied post-matmul (sigmoid's scale operand +
tensor_scalar ops) — the PE supports mixed fp8-stationary x fp16-moving
matmuls at full fp16 speed, verified bit-exact on HW.

The scales gate_s*up_s span ~100x, so output error is dominated by the
loudest intermediate channels. Channels are permuted host-side by
descending gate_s*up_s: the loudest NL=16 i-tiles keep x and hdn in fp16
(error ~= fp16 baseline); the quiet NQ=28 i-tiles use fp8(x)/fp8(hdn)
with DoubleRow matmuls (2 k-tiles per instruction = 2x PE throughput).
Simulated absmax rel err 1.64e-2 vs the 2e-2 gate (HW matched sim to
5e-5 at NL=18).

Per core, everything is computed in "transposed" space so the contraction
dim always sits on SBUF partitions:
  phase 1: for each inter tile  g^T/u^T [128i, 512t] = W-tile.T @ x^T
           hdn^T = sigmoid(gs*g)*g * (gs*us*u)  (fp16 loud / fp8 quiet)
  phase 2: for each of 16 hidden tiles out^T [128h, 512t] = D-tile.T @ hdn^T
           scaled by down_s per partition.
"""

import numpy as np
import ml_dtypes

HIDDEN = 2048
INTER = 5632
B, S = 2, 2048
T = B * S
NCORES = 8
TPC = T // NCORES          # 512 tokens per core
P = 128
KH = HIDDEN // P           # 16 hidden-dim k-tiles
NI = INTER // P            # 44 intermediate tiles
NL = 16                    # loud (fp16 activations) i-tiles
NQ = NI - NL               # quiet (fp8 DoubleRow) i-tiles; must be even

_cache = {}


def _build_nc(kh=KH, nl=NL, nq=NQ, ni=NI, tpc=TPC):
    import concourse.bacc as bacc
    import concourse.tile as tile
    from concourse import mybir

    f16 = mybir.dt.float16
    f32 = mybir.dt.float32
    f8 = mybir.dt.float8e4
    DR = mybir.MatmulPerfMode.DoubleRow

    nc = bacc.Bacc("TRN2", target_bir_lowering=False, debug=False)
    xt = nc.dram_tensor("xt", [P, kh, tpc], f16, kind="ExternalInput").ap()
    xt8 = nc.dram_tensor("xt8", [P, kh, tpc], f8, kind="ExternalInput").ap()
    gw8 = nc.dram_tensor("gw8", [ni, P, kh, P], f8, kind="ExternalInput").ap()
    uw8 = nc.dram_tensor("uw8", [ni, P, kh, P], f8, kind="ExternalInput").ap()
    dw8 = nc.dram_tensor("dw8", [kh, P, ni, P], f8, kind="ExternalInput").ap()
    gsa = nc.dram_tensor("gsa", [P, ni], f32, kind="ExternalInput").ap()
    gua = nc.dram_tensor("gua", [P, ni], f32, kind="ExternalInput").ap()
    dst = nc.dram_tensor("dst", [P, kh], f32, kind="ExternalInput").ap()
    ot = nc.dram_tensor("ot", [kh, P, tpc], f16, kind="ExternalOutput").ap()

    with tile.TileContext(nc) as tc:
        with (
            tc.tile_pool(name="xp", bufs=1) as xp,
            tc.tile_pool(name="hp", bufs=1) as hp,
            tc.tile_pool(name="sp", bufs=1) as sp,
            tc.tile_pool(name="wg", bufs=3) as wg_pool,
            tc.tile_pool(name="wu", bufs=3) as wu_pool,
            tc.tile_pool(name="wd", bufs=4) as wd_pool,
            tc.tile_pool(name="act", bufs=3) as act_pool,
            tc.tile_pool(name="ob", bufs=3) as ob_pool,
            tc.tile_pool(name="ps", bufs=2, space="PSUM") as ps_pool,
            tc.tile_pool(name="po", bufs=2, space="PSUM") as po_pool,
        ):
            # PE warmup: HAM un-throttles only after ~3.4us of sustained PE
            # activity; bridge the initial DMA wait with dummy matmuls on a
            # zeroed tile so the real stream starts at 2.4 GHz.
            wz = act_pool.tile([P, P], f16, tag="warm")
            nc.vector.memset(wz[:], 0.0)
            pw = po_pool.tile([P, P], f32, tag="warmp")
            for _ in range(20):
                nc.tensor.matmul(pw[:], wz[:], wz[:], start=True, stop=True)

            xsb = xp.tile([P, kh, tpc], f16)
            x8sb = xp.tile([P, kh, tpc], f8, tag="x8")
            hdl = hp.tile([P, nl, tpc], f16, tag="hl")
            hdq = hp.tile([P, nq, tpc], f8, tag="hq")
            gsa_sb = sp.tile([P, ni], f32, tag="gsa")
            gua_sb = sp.tile([P, ni], f32, tag="gua")
            dst_sb = sp.tile([P, kh], f32, tag="dst")
            nc.sync.dma_start(out=gsa_sb[:], in_=gsa)
            nc.sync.dma_start(out=gua_sb[:], in_=gua)
            nc.sync.dma_start(out=dst_sb[:], in_=dst)

            # issue the first loads in fine-grained consumption order so the
            # first real matmul can start as early as possible: first 4
            # k-tiles of gate slab 0 / x / up slab 0, then the remainders,
            # then fp8 x
            ck = max(1, kh // 4)
            wgt0 = wg_pool.tile([P, kh, P], f8, tag="wgt")
            wut0 = wu_pool.tile([P, kh, P], f8, tag="wut")
            nc.sync.dma_start(out=wgt0[:, 0:ck], in_=gw8[0][:, 0:ck])
            nc.sync.dma_start(out=xsb[:, 0:ck], in_=xt[:, 0:ck])
            nc.sync.dma_start(out=wut0[:, 0:ck], in_=uw8[0][:, 0:ck])
            nc.sync.dma_start(out=wgt0[:, ck:], in_=gw8[0][:, ck:])
            nc.sync.dma_start(out=xsb[:, ck:2 * ck], in_=xt[:, ck:2 * ck])
            nc.sync.dma_start(out=wut0[:, ck:], in_=uw8[0][:, ck:])
            for lo in range(2 * ck, kh, ck):
                nc.sync.dma_start(out=xsb[:, lo:lo + ck], in_=xt[:, lo:lo + ck])
            nc.sync.dma_start(out=x8sb[:], in_=xt8)

            for it in range(ni):
                if it == 0:
                    wgt, wut = wgt0, wut0
                else:
                    wgt = wg_pool.tile([P, kh, P], f8, tag="wgt")
                    nc.sync.dma_start(out=wgt[:], in_=gw8[it])
                    wut = wu_pool.tile([P, kh, P], f8, tag="wut")
                    nc.sync.dma_start(out=wut[:], in_=uw8[it])
                pg = ps_pool.tile([P, tpc], f32)
                pu = ps_pool.tile([P, tpc], f32)
                if it < nl:
                    # loud: fp16 x moving, fp8 ternary stationary (exact)
                    for k in range(kh):
                        nc.tensor.matmul(
                            pg[:], wgt[:, k], xsb[:, k],
                            start=(k == 0), stop=(k == kh - 1),
                        )
                    for k in range(kh):
                        nc.tensor.matmul(
                            pu[:], wut[:, k], xsb[:, k],
                            start=(k == 0), stop=(k == kh - 1),
                        )
                else:
                    # quiet: fp8 x, DoubleRow over k-tile pairs
                    for k in range(0, kh, 2):
                        nc.tensor.matmul(
                            pg[:], wgt[:, k:k + 2], x8sb[:, k:k + 2],
                            start=(k == 0), stop=(k == kh - 2), perf_mode=DR,
                        )
                    for k in range(0, kh, 2):
                        nc.tensor.matmul(
                            pu[:], wut[:, k:k + 2], x8sb[:, k:k + 2],
                            start=(k == 0), stop=(k == kh - 2), perf_mode=DR,
                        )
                # hdn = silu(gs*g)*us*u = sigmoid(gs*g) * g * (gs*us*u)
                sg = act_pool.tile([P, tpc], f16)
                nc.scalar.activation(sg[:], pg[:],
                                     mybir.ActivationFunctionType.Sigmoid,
                                     scale=gsa_sb[:, it:it + 1])
                tq = act_pool.tile([P, tpc], f16)
                nc.vector.tensor_mul(tq[:], sg[:], pg[:])
                usc = act_pool.tile([P, tpc], f16)
                nc.vector.tensor_scalar_mul(usc[:], pu[:], gua_sb[:, it:it + 1])
                if it < nl:
                    nc.vector.tensor_mul(hdl[:, it], tq[:], usc[:])
                else:
                    nc.vector.tensor_mul(hdq[:, it - nl], tq[:], usc[:])

            for hg in range(kh):
                wdt = wd_pool.tile([P, ni, P], f8, tag="wd")
                nc.sync.dma_start(out=wdt[:], in_=dw8[hg])
                po = po_pool.tile([P, tpc], f32)
                for il in range(nl):
                    nc.tensor.matmul(
                        po[:], wdt[:, il], hdl[:, il],
                        start=(il == 0), stop=False,
                    )
                for qj in range(0, nq, 2):
                    nc.tensor.matmul(
                        po[:], wdt[:, nl + qj:nl + qj + 2], hdq[:, qj:qj + 2],
                        start=False, stop=(qj == nq - 2), perf_mode=DR,
                    )
                ob = ob_pool.tile([P, tpc], f16)
                nc.vector.tensor_scalar_mul(ob[:], po[:], dst_sb[:, hg:hg + 1])
                nc.sync.dma_start(out=ot[hg], in_=ob[:])

    nc.compile()
    return nc


def _pack_weights(gate_w, up_w, down_w, gate_s, up_s, down_s):
    f8 = ml_dtypes.float8_e4m3
    perm = np.argsort(-(gate_s * up_s), kind="stable")
    gw = gate_w[perm]
    uw = up_w[perm]
    dw = down_w[:, perm]
    gs = gate_s[perm]
    us = up_s[perm]

    def pack_gu(w):
        wp = w.reshape(NI, P, KH, P).transpose(0, 3, 2, 1)
        return np.ascontiguousarray(wp).astype(f8)

    gw8 = pack_gu(gw)
    uw8 = pack_gu(uw)
    dwp = dw.reshape(KH, P, NI, P).transpose(0, 3, 2, 1)
    dw8 = np.ascontiguousarray(dwp).astype(f8)

    gsa = np.ascontiguousarray(gs.reshape(NI, P).T)
    gua = np.ascontiguousarray((gs * us).reshape(NI, P).T)
    dst = np.ascontiguousarray(down_s.reshape(KH, P).T)
    return dict(gw8=gw8, uw8=uw8, dw8=dw8, gsa=gsa, gua=gua, dst=dst)


def _pack_x(xf):
    # per-core x^T tiles: xt[p, k, t] = x_core[t, k*128+p]
    outs = []
    for c in range(NCORES):
        xc = xf[c * TPC:(c + 1) * TPC].reshape(TPC, KH, P)
        xc = np.ascontiguousarray(xc.transpose(2, 1, 0))
        outs.append((xc.astype(np.float16),
                     xc.astype(ml_dtypes.float8_e4m3)))
    return outs


def _ensure_ntff_hook():
    """bass_utils' axon trace path imports antenv.axon_hooks, which is
    missing from this image; provide it (ctypes into libaxon_pjrt.so) so a
    BASS_TRACE=1 environment doesn't crash the run."""
    import sys
    try:
        import antenv.axon_hooks  # noqa: F401
        return
    except ImportError:
        pass
    import contextlib
    import ctypes
    import types

    def _make_hook():
        try:
            lib = ctypes.CDLL("/opt/axon/libaxon_pjrt.so")
            lib.axon_start_nrt_profile
        except Exception:
            return None
        lib.axon_start_nrt_profile.argtypes = [ctypes.POINTER(ctypes.c_int64),
                                               ctypes.c_size_t]
        lib.axon_start_nrt_profile.restype = ctypes.c_int64
        lib.axon_stop_nrt_profile.argtypes = [ctypes.c_char_p]
        lib.axon_stop_nrt_profile.restype = ctypes.c_int64

        @contextlib.contextmanager
        def _hook(output_dir, device_ids):
            import jax
            jax.devices()
            if device_ids:
                ids = (ctypes.c_int64 * len(device_ids))(*device_ids)
                rc = lib.axon_start_nrt_profile(ids, len(device_ids))
            else:
                rc = lib.axon_start_nrt_profile(None, 0)
            if rc != 0:
                raise RuntimeError(f"axon_start_nrt_profile rc={rc}")
            try:
                yield
            finally:
                lib.axon_stop_nrt_profile(str(output_dir).encode())

        return _hook

    mod = types.ModuleType("antenv.axon_hooks")
    _hook = _make_hook()
    mod.get_axon_ntff_profile_hook = lambda: _hook
    mod.set_axon_ntff_profile_hook = lambda h: None
    sys.modules["antenv.axon_hooks"] = mod


def _run(in_maps, trace=False, tmpdir=None, trace_cores=None):
    from concourse.bass_utils import run_bass_kernel_spmd

    _ensure_ntff_hook()
    if "nc" not in _cache:
        _cache["nc"] = _build_nc()
    return run_bass_kernel_spmd(
        _cache["nc"], in_maps, list(range(NCORES)), trace=trace, tmpdir=tmpdir,
        trace_cores=trace_cores,
    )


def make_in_maps(x, gate_w, up_w, down_w, gate_s, up_s, down_s):
    x = np.asarray(x, np.float32)
    gate_w = np.asarray(gate_w, np.float32)
    up_w = np.asarray(up_w, np.float32)
    down_w = np.asarray(down_w, np.float32)
    gate_s = np.asarray(gate_s, np.float32)
    up_s = np.asarray(up_s, np.float32)
    down_s = np.asarray(down_s, np.float32)

    wmap = _pack_weights(gate_w, up_w, down_w, gate_s, up_s, down_s)
    xts = _pack_x(x.reshape(T, HIDDEN))
    return [dict(xt=xts[c][0], xt8=xts[c][1], **wmap) for c in range(NCORES)]


def unpack_out(results):
    # ot: (16, 128, 512) f16 per core; out_core[t, hg*128+p] = ot[hg, p, t]
    parts = []
    for c in range(NCORES):
        ot = results[c]["ot"].astype(np.float32)
        parts.append(ot.transpose(2, 0, 1).reshape(TPC, HIDDEN))
    return np.concatenate(parts, axis=0).reshape(B, S, HIDDEN)


def kernel(x, gate_w, up_w, down_w, gate_s, up_s, down_s):
    in_maps = make_in_maps(x, gate_w, up_w, down_w, gate_s, up_s, down_s)
    try:
        res = _run(in_maps)
    except Exception:
        # transient runtime errors (device hiccup) — one retry
        res = _run(in_maps)
    return unpack_out(res.results)
